# revision 46
# baseline (speedup 1.0000x reference)
"""Trainium2 Bass kernel: AttentionBlock (GroupNorm + 1x1-conv QKV + MHA + proj + residual).

Data-parallel over batch: 16 samples -> 8 NeuronCores x 2 samples. Each core
runs the whole block locally (attention is per-sample, no collectives); the
host shards inputs and concatenates the 8 output shards.

Math notes (exact rewrites, not approximations):
  - scores are computed transposed, S^T[m,n] = sum_d k[d,m] q'[d,n] with
    q' = (q + b_q) * d^-0.5. The k-bias adds a column-constant to S^T which
    softmax cancels, so it is dropped.
  - softmax denominator comes from a ones-column appended to v^T in the
    attn@v matmul (row 64 of the [65, n] output accumulates colsum(exp S^T)).
  - v-bias: attn rows sum to 1, so  attn @ (Wv h + bv) = attn @ Wv h + bv;
    the bv term is folded into the proj bias on the host:
    beff = b_proj + w_proj @ bv.
"""

import os
from contextlib import ExitStack

import ml_dtypes
import numpy as np

import concourse.bass as bass
import concourse.tile as tile
from concourse import bacc
from concourse import mybir
from concourse.bass_utils import run_bass_kernel_spmd

F32 = mybir.dt.float32
BF16 = mybir.dt.bfloat16
AF = mybir.ActivationFunctionType
ALU = mybir.AluOpType

# Problem dims (hardcoded per spec: x [16, 512, 32, 32] f32)
B, C, H, W = 16, 512, 32, 32
N = H * W                # 1024 spatial positions
NCORES = 8
BS = B // NCORES         # 2 samples per core
G = 32                   # groupnorm groups
HEADS = 8
D = C // HEADS           # 64
CT = C // 128            # 4 channel tiles
MT = N // 128            # 8 m-tiles (spatial, attention contraction)
NHALF = 2                # n split in halves of 512 (psum bank limit)
EPS = 1e-5
GROUP_ELEMS = (C // G) * N   # 16 ch * 1024 = 16384 per group

LAST_EXEC_NS = None
LAST_RESULTS = None


def _build_tile(ctx: ExitStack, tc: tile.TileContext, te: dict):
    nc = tc.nc
    x_e, out_e = te["x"], te["out"]

    const = ctx.enter_context(tc.tile_pool(name="const", bufs=1))
    small = ctx.enter_context(tc.tile_pool(name="small", bufs=6))
    ps_acc = ctx.enter_context(tc.tile_pool(name="ps_acc", bufs=4, space="PSUM"))
    ps_sc = ctx.enter_context(tc.tile_pool(name="ps_sc", bufs=2, space="PSUM"))

    # ---- groupnorm stats over the [BS*G, 16384] view of x ----
    # Issued before the weight loads: the stats chain gates the first matmul.
    # Chunked DMAs so bn_stats tracks the stream instead of one 4MB barrier.
    NCHUNK = GROUP_ELEMS // 512          # bn_stats hw max free = 512
    GCH = 4
    stats_sb = const.tile([BS * G, NCHUNK, 6], F32)
    eps_sb = const.tile([BS * G, 1], F32)
    nc.vector.memset(eps_sb[:], EPS)
    # preload the Exp ACT table set off the critical path
    dummy_act = const.tile([1, 1], F32)
    nc.scalar.activation(dummy_act[:], eps_sb[0:1, :], AF.Exp)
    # stats on [128, 8192] half-group rows: full-width DMA ports (a [64, N]
    # layout would halve DMA bandwidth) and half the bn_stats calls
    HSUB = GROUP_ELEMS // 2 // 512 // GCH  # 512-wide bn_stats per DMA chunk
    stats2 = const.tile([128, GROUP_ELEMS // 2 // 512, 6], F32)
    with tc.tile_pool(name="gnx", bufs=2) as gnxp:
        for gc in range(GCH):
            gnx = gnxp.tile([128, HSUB, 512], F32, tag="gnx", name="gnx")
            in_ap = bass.AP(
                tensor=x_e,
                offset=gc * HSUB * 512,
                ap=[[C * N, BS], [GROUP_ELEMS // 2, 2 * G], [1, HSUB * 512]],
            )
            nc.sync.dma_start(out=gnx[:], in_=in_ap)
            for j in range(HSUB):
                nc.vector.bn_stats(out=stats2[:, gc * HSUB + j, :], in_=gnx[:, j, :])
    # fold half-group stats rows back to [group, 2*chunks] (both sides are
    # contiguous, single sbuf-to-sbuf DMA), then aggregate per group
    nc.gpsimd.dma_start(out=stats_sb[:], in_=stats2[:])

    # ---- constants / weights to SBUF (needed ~30us in; loads overlap stats) ----
    wqk_sb = const.tile([128, CT, 2 * C], BF16)   # w_qkv[:1024].T tiles
    wv_sb = const.tile([128, CT, C], BF16)        # w_qkv[1024:].T tiles
    wp_sb = const.tile([128, CT, C], BF16)        # w_proj.T tiles
    bq_sb = const.tile([128, CT, 1], F32)
    beff_sb = const.tile([128, CT, 1], F32)
    for kt in range(CT):
        sl = slice(kt * 128, (kt + 1) * 128)
        nc.sync.dma_start(out=wqk_sb[:, kt, :], in_=te["wqkT"][sl, :])
        nc.sync.dma_start(out=wv_sb[:, kt, :], in_=te["wvT"][sl, :])
        nc.sync.dma_start(out=wp_sb[:, kt, :], in_=te["wpT"][sl, :])
        nc.sync.dma_start(out=bq_sb[:, kt, :], in_=te["bq"][sl, :])
        nc.sync.dma_start(out=beff_sb[:, kt, :], in_=te["beff"][sl, :])
    # gamma/beta replicated per sample: [128, (s, t)] layout
    gam2 = const.tile([128, BS * CT], F32)
    bet2 = const.tile([128, BS * CT], F32)
    for s in range(BS):
        nc.sync.dma_start(
            out=gam2[:, s * CT : (s + 1) * CT],
            in_=bass.AP(tensor=te["gamma"], offset=0, ap=[[1, 128], [128, CT]]),
        )
        nc.sync.dma_start(
            out=bet2[:, s * CT : (s + 1) * CT],
            in_=bass.AP(tensor=te["beta"], offset=0, ap=[[1, 128], [128, CT]]),
        )

    mv = small.tile([BS * G, 2], F32, tag="mv")
    nc.vector.bn_aggr(out=mv[:], in_=stats_sb[:])
    # rstd = rsqrt(var + eps) via int-seed + 2 Newton steps, all on DVE —
    # keeps ACT on the single Exp table set for the whole kernel (Ln/Sqrt
    # would force table reloads).
    I32 = mybir.dt.int32
    st2 = small.tile([BS * G, 2], mybir.dt.float32r, tag="st2")
    nc.vector.tensor_copy(st2[:, 0:1], mv[:, 0:1])
    vpe = small.tile([BS * G, 1], F32, tag="vpe")
    nc.vector.tensor_scalar_add(vpe[:], mv[:, 1:2], EPS)
    hv = small.tile([BS * G, 1], F32, tag="hv")
    nc.vector.tensor_scalar_mul(hv[:], vpe[:], -0.5)
    y0 = small.tile([BS * G, 1], F32, tag="y0")
    ysh = small.tile([BS * G, 1], I32, tag="ysh")
    nc.vector.tensor_scalar(
        out=ysh[:],
        in0=vpe[:].bitcast(I32),
        scalar1=1,
        scalar2=None,
        op0=ALU.arith_shift_right,
    )
    nc.vector.tensor_scalar(
        out=y0[:].bitcast(I32),
        in0=ysh[:],
        scalar1=-1,
        scalar2=0x5F3759DF,
        op0=ALU.mult,
        op1=ALU.add,
    )
    y1 = small.tile([BS * G, 1], F32, tag="y1")
    yw = small.tile([BS * G, 1], F32, tag="yw")
    # Newton: y <- y * (1.5 - 0.5*v*y^2), twice
    nc.vector.tensor_mul(yw[:], y0[:], y0[:])
    nc.vector.tensor_mul(yw[:], yw[:], hv[:])
    nc.vector.tensor_scalar_add(yw[:], yw[:], 1.5)
    nc.vector.tensor_mul(y1[:], y0[:], yw[:])
    nc.vector.tensor_mul(yw[:], y1[:], y1[:])
    nc.vector.tensor_mul(yw[:], yw[:], hv[:])
    nc.vector.tensor_scalar_add(yw[:], yw[:], 1.5)
    nc.vector.tensor_mul(st2[:, 1:2], y1[:], yw[:])

    # broadcast group stats to channel vectors with tiny selector matmuls on
    # the (otherwise idle) PE: mvr[p, j, :] = (mean, rstd) of group g(p, j);
    # SEL comes from the host, f32r keeps the stats at ~f32 precision
    F32R = mybir.dt.float32r
    A_all = const.tile([128, BS * CT], F32)
    B_all = const.tile([128, BS * CT], F32)
    with tc.tile_pool(name="selp", bufs=1) as selp:
        sel_sb = selp.tile([BS * G, BS * CT, 128], F32R)
        nc.sync.dma_start(
            out=sel_sb[:], in_=te["sel"][:].rearrange("g (j p) -> g j p", p=128)
        )
        mvr_ps = ps_sc.tile([128, BS * CT, 2], F32, tag="sc", name="mvr_ps")
        for j in range(BS * CT):
            nc.tensor.matmul(
                mvr_ps[:, j, :],
                sel_sb[:, j, :],
                st2[:],
                start=True,
                stop=True,
            )
        # h = x*A + Bv over all (s, t): A = rstd*gamma, Bv = beta - mean*A
        nc.vector.tensor_mul(A_all[:], mvr_ps[:, :, 1], gam2[:])
        tmpA = small.tile([128, BS * CT], F32, tag="tmpA")
        nc.vector.tensor_mul(tmpA[:], mvr_ps[:, :, 0], A_all[:])
        nc.vector.tensor_sub(B_all[:], bet2[:], tmpA[:])

    # Main pools open after the gn-stats/sel pools have freed their space.
    xpool = ctx.enter_context(tc.tile_pool(name="xres", bufs=1))
    hpool = ctx.enter_context(tc.tile_pool(name="h", bufs=1))
    qkpool = ctx.enter_context(tc.tile_pool(name="qk", bufs=1))
    vtpool = ctx.enter_context(tc.tile_pool(name="vt", bufs=2))
    atpool = ctx.enter_context(tc.tile_pool(name="attn", bufs=2))
    aopool = ctx.enter_context(tc.tile_pool(name="ao", bufs=2))
    rbpool = ctx.enter_context(tc.tile_pool(name="rb", bufs=4))
    rcppool = ctx.enter_context(tc.tile_pool(name="rcps", bufs=2))
    expipool = ctx.enter_context(tc.tile_pool(name="expi", bufs=1))
    outpool = ctx.enter_context(tc.tile_pool(name="outp", bufs=2))

    x_sb = xpool.tile([128, BS * CT, N], F32)
    for s in range(BS):
        for t in range(CT):
            nc.gpsimd.dma_start(
                out=x_sb[:, s * CT + t, :], in_=x_e[s, t * 128 : (t + 1) * 128, :]
            )

    def emit_prep(s):
        # ---- groupnorm apply -> h (bf16), then qk / vT matmuls ----
        h_sb = hpool.tile([128, CT, N], BF16, tag="h", name="h_sb")
        for t in range(CT):
            nc.vector.tensor_scalar(
                out=h_sb[:, t, :],
                in0=x_sb[:, s * CT + t, :],
                scalar1=A_all[:, s * CT + t : s * CT + t + 1],
                scalar2=B_all[:, s * CT + t : s * CT + t + 1],
                op0=ALU.mult,
                op1=ALU.add,
            )

        # ---- qk = wqkT.T @ h   ([o, n], o-tile p holds heads 2p, 2p+1) ----
        q_sb = qkpool.tile([128, CT, N], BF16, tag="q", name="q_sb")
        k_sb = qkpool.tile([128, CT, N], BF16, tag="k", name="k_sb")
        for o in range(2 * CT):
            for nh in range(NHALF):
                ps = ps_acc.tile([128, 512], F32, tag="acc", name="ps")
                for kt in range(CT):
                    nc.tensor.matmul(
                        ps[:],
                        wqk_sb[:, kt, o * 128 : (o + 1) * 128],
                        h_sb[:, kt, nh * 512 : (nh + 1) * 512],
                        start=(kt == 0),
                        stop=(kt == CT - 1),
                    )
                if o < CT:  # q channels: scale+bias fused into the copy
                    nc.vector.tensor_scalar(
                        out=q_sb[:, o, nh * 512 : (nh + 1) * 512],
                        in0=ps[:],
                        scalar1=bq_sb[:, o, :],
                        scalar2=float(D) ** -0.5,
                        op0=ALU.add,
                        op1=ALU.mult,
                    )
                else:  # k channels: plain copy (bias dropped, see header)
                    nc.vector.tensor_copy(
                        k_sb[:, o - CT, nh * 512 : (nh + 1) * 512], ps[:]
                    )

        # ---- vT = h.T @ wvT  ([m, dv] + ones column for colsum) ----
        vt_sb = vtpool.tile([128, MT, HEADS, D + 1], BF16, tag="vt")
        nc.vector.memset(vt_sb[:, :, :, D : D + 1], 1.0)
        for m in range(MT):
            ps = ps_acc.tile([128, 512], F32, tag="acc", name="ps")
            for kt in range(CT):
                nc.tensor.matmul(
                    ps[:],
                    h_sb[:, kt, m * 128 : (m + 1) * 128],
                    wv_sb[:, kt, :],
                    start=(kt == 0),
                    stop=(kt == CT - 1),
                )
            nc.vector.tensor_copy(
                vt_sb[:, m, :, 0:D], ps[:].rearrange("p (h d) -> p h d", h=HEADS)
            )

        return q_sb, k_sb, vt_sb

    def emit_attention(s, q_sb, k_sb, vt_sb):
        # ---- attention: QK/exp of pair p interleaved with AV of pair p-1 ----
        # (fills the PE gaps while ACT runs exp; ~2x denser PE stream)
        ao_sb = aopool.tile([128, CT, N], BF16, tag="ao", name="ao_sb")

        def emit_av_chunk(prev_state, m):
            p0, at0, avs0 = prev_state
            for hh in range(2):
                for nh in range(NHALF):
                    nc.tensor.matmul(
                        avs0[hh][nh][:],
                        vt_sb[:, m, 2 * p0 + hh, :],
                        at0[:, hh, m, nh * 512 : (nh + 1) * 512],
                        start=(m == 0),
                        stop=(m == MT - 1),
                    )

        def emit_normalize(prev_state):
            p0, at0, avs0 = prev_state
            for hh in range(2):
                for nh in range(NHALF):
                    nsl = slice(nh * 512, (nh + 1) * 512)
                    # custom-DVE recip misreads PSUM sources on HW: SBUF-bounce
                    cs = rcppool.tile([1, 512], F32, tag="cs", name="cs")
                    nc.vector.tensor_copy(cs[:], avs0[hh][nh][D : D + 1, :])
                    rcp = rcppool.tile([1, 512], F32, tag="rcp", name="rcp")
                    nc.vector.reciprocal_approx_fast(rcp[:], cs[:])
                    rb = rbpool.tile([64, 512], F32, tag="rb", name="rb")
                    nc.gpsimd.partition_broadcast(rb[:], rcp[:])
                    nc.vector.tensor_mul(
                        ao_sb[hh * 64 : (hh + 1) * 64, p0, nsl],
                        avs0[hh][nh][0:D, :],
                        rb[:],
                    )

        prev = None
        for p in range(HEADS // 2):
            at_pair = atpool.tile([128, 2, MT, N], BF16, tag="attn", name="at_pair")
            for m in range(MT):
                for hh in range(2):
                    base = hh * 64
                    sc = ps_sc.tile([128, N], F32, tag="sc", name="sc")
                    for nh in range(NHALF):
                        nsl = slice(nh * 512, (nh + 1) * 512)
                        nc.tensor.matmul(
                            sc[:, nsl],
                            k_sb[base : base + 64, p, m * 128 : (m + 1) * 128],
                            q_sb[base : base + 64, p, nsl],
                            start=True,
                            stop=True,
                            tile_position=(base, 0),
                        )
                    if hh == 1 and m % 2 == 1:
                        # DVE Schraudolph exp: rebalances the attention phase
                        # off the ACT engine (softmax renorm absorbs the
                        # ~1-3% per-element approx error mostly)
                        ei = expipool.tile([128, N], I32, tag="ei", name="ei")
                        nc.vector.tensor_scalar(
                            out=ei[:],
                            in0=sc[:],
                            scalar1=12102203.161561485,
                            scalar2=1064866805.0,
                            op0=ALU.mult,
                            op1=ALU.add,
                        )
                        nc.vector.tensor_copy(
                            at_pair[:, hh, m, :], ei[:].bitcast(F32)
                        )
                    else:
                        nc.scalar.activation(at_pair[:, hh, m, :], sc[:], AF.Exp)
                if prev is not None:
                    emit_av_chunk(prev, m)
            if prev is not None:
                emit_normalize(prev)
            if p < HEADS // 2 - 1:
                avs = [
                    [
                        ps_acc.tile([D + 1, 512], F32, tag="acc", name=f"av{hh}_{nh}")
                        for nh in range(NHALF)
                    ]
                    for hh in range(2)
                ]
            else:
                # drain pair accumulates in the (then idle) scores pool so the
                # acc pool frees for the next sample's qkv before normalize
                dr = [
                    ps_sc.tile([D + 1, N], F32, tag="sc", name=f"drain{hh}")
                    for hh in range(2)
                ]
                avs = [
                    [dr[hh][:, nh * 512 : (nh + 1) * 512] for nh in range(NHALF)]
                    for hh in range(2)
                ]
            prev = (p, at_pair, avs)
        for m in range(MT):
            emit_av_chunk(prev, m)
        emit_normalize(prev)
        return ao_sb

    def emit_proj(s, ao_sb):
        # ---- proj + bias + residual ----
        for t in range(CT):
            for nh in range(NHALF):
                nsl = slice(nh * 512, (nh + 1) * 512)
                ps = ps_acc.tile([128, 512], F32, tag="acc", name="ps")
                for kt in range(CT):
                    nc.tensor.matmul(
                        ps[:],
                        wp_sb[:, kt, t * 128 : (t + 1) * 128],
                        ao_sb[:, kt, nsl],
                        start=(kt == 0),
                        stop=(kt == CT - 1),
                    )
                ot = outpool.tile([128, 512], F32, tag="out")
                nc.vector.scalar_tensor_tensor(
                    out=ot[:],
                    in0=ps[:],
                    scalar=beff_sb[:, t, :],
                    in1=x_sb[:, s * CT + t, nsl],
                    op0=ALU.add,
                    op1=ALU.add,
                )
                nc.gpsimd.dma_start(
                    out=out_e[s, t * 128 : (t + 1) * 128, nsl], in_=ot[:]
                )

    # Drive: emit next sample's qkv prep between a sample's attention drain
    # and its proj, so the PE instruction stream has work while the
    # normalize (recip -> DRAM bounce -> broadcast) latency resolves.
    tiles = emit_prep(0)
    for s in range(BS):
        ao = emit_attention(s, *tiles)
        if s + 1 < BS:
            tiles = emit_prep(s + 1)
        emit_proj(s, ao)


def build_bass() -> bass.Bass:
    nc = bacc.Bacc()
    te = {
        "x": nc.declare_dram_parameter("x", [BS, C, N], F32, isOutput=False),
        "wqkT": nc.declare_dram_parameter("wqkT", [C, 2 * C], BF16, isOutput=False),
        "wvT": nc.declare_dram_parameter("wvT", [C, C], BF16, isOutput=False),
        "wpT": nc.declare_dram_parameter("wpT", [C, C], BF16, isOutput=False),
        "bq": nc.declare_dram_parameter("bq", [C, 1], F32, isOutput=False),
        "beff": nc.declare_dram_parameter("beff", [C, 1], F32, isOutput=False),
        "gamma": nc.declare_dram_parameter("gamma", [C, 1], F32, isOutput=False),
        "beta": nc.declare_dram_parameter("beta", [C, 1], F32, isOutput=False),
        "sel": nc.declare_dram_parameter(
            "sel", [BS * G, BS * CT * 128], mybir.dt.float32r, isOutput=False
        ),
        "out": nc.declare_dram_parameter("out", [BS, C, N], F32, isOutput=True),
    }
    with tile.TileContext(nc) as tc:
        with ExitStack() as ctx:
            _build_tile(ctx, tc, te)
    # Bacc defers register allocation to finalize(); run_bass_via_pjrt
    # serializes the module without calling it, so do it here.
    nc.finalize()
    return nc


def _make_sel() -> np.ndarray:
    sel = np.zeros((BS * G, BS * CT, 128), np.float32)
    for j in range(BS * CT):
        s0, t0 = j // CT, j % CT
        for p in range(128):
            sel[s0 * G + t0 * 8 + p // 16, j, p] = 1.0
    return sel.reshape(BS * G, BS * CT * 128)


def make_in_maps(inputs: dict) -> list[dict]:
    x = np.ascontiguousarray(np.asarray(inputs["x"], np.float32)).reshape(B, C, N)
    w_qkv = np.asarray(inputs["w_qkv"], np.float32)
    b_qkv = np.asarray(inputs["b_qkv"], np.float32)
    w_proj = np.asarray(inputs["w_proj"], np.float32)
    b_proj = np.asarray(inputs["b_proj"], np.float32)
    gamma = np.asarray(inputs["gamma"], np.float32)
    beta = np.asarray(inputs["beta"], np.float32)

    bf = ml_dtypes.bfloat16
    common = {
        "wqkT": np.ascontiguousarray(w_qkv[: 2 * C, :].T).astype(bf),
        "wvT": np.ascontiguousarray(w_qkv[2 * C :, :].T).astype(bf),
        "wpT": np.ascontiguousarray(w_proj.T).astype(bf),
        "bq": b_qkv[:C].reshape(C, 1).copy(),
        "beff": (b_proj + w_proj @ b_qkv[2 * C :]).reshape(C, 1).astype(np.float32),
        "gamma": gamma.reshape(C, 1).copy(),
        "beta": beta.reshape(C, 1).copy(),
        "sel": _make_sel(),
    }
    return [
        {"x": np.ascontiguousarray(x[i * BS : (i + 1) * BS]), **common}
        for i in range(NCORES)
    ]


def kernel(**inputs) -> np.ndarray:
    global LAST_EXEC_NS, LAST_RESULTS
    nc = build_bass()
    in_maps = make_in_maps(inputs)
    res = run_bass_kernel_spmd(nc, in_maps, list(range(NCORES)))
    LAST_RESULTS = res
    LAST_EXEC_NS = res.exec_time_ns
    out = np.concatenate([np.asarray(res.results[i]["out"]) for i in range(NCORES)], 0)
    return out.reshape(B, C, H, W).astype(np.float32)


# revision 47
# speedup vs baseline: 1.1125x; 1.1125x over previous
"""Trainium2 Bass kernel: AttentionBlock (GroupNorm + 1x1-conv QKV + MHA + proj + residual).

Data-parallel over batch: 16 samples -> 8 NeuronCores x 2 samples. Each core
runs the whole block locally (attention is per-sample, no collectives); the
host shards inputs and concatenates the 8 output shards.

Math notes (exact rewrites, not approximations):
  - scores are computed transposed, S^T[m,n] = sum_d k[d,m] q'[d,n] with
    q' = (q + b_q) * d^-0.5. The k-bias adds a column-constant to S^T which
    softmax cancels, so it is dropped.
  - softmax denominator comes from a ones-column appended to v^T in the
    attn@v matmul (row 64 of the [65, n] output accumulates colsum(exp S^T)).
  - v-bias: attn rows sum to 1, so  attn @ (Wv h + bv) = attn @ Wv h + bv;
    the bv term is folded into the proj bias on the host:
    beff = b_proj + w_proj @ bv.
"""

import os
from contextlib import ExitStack

import ml_dtypes
import numpy as np

import concourse.bass as bass
import concourse.tile as tile
from concourse import bacc
from concourse import mybir
from concourse.bass_utils import run_bass_kernel_spmd

F32 = mybir.dt.float32
BF16 = mybir.dt.bfloat16
AF = mybir.ActivationFunctionType
ALU = mybir.AluOpType

# Problem dims (hardcoded per spec: x [16, 512, 32, 32] f32)
B, C, H, W = 16, 512, 32, 32
N = H * W                # 1024 spatial positions
NCORES = 8
BS = B // NCORES         # 2 samples per core
G = 32                   # groupnorm groups
HEADS = 8
D = C // HEADS           # 64
CT = C // 128            # 4 channel tiles
MT = N // 128            # 8 m-tiles (spatial, attention contraction)
NHALF = 2                # n split in halves of 512 (psum bank limit)
EPS = 1e-5
GROUP_ELEMS = (C // G) * N   # 16 ch * 1024 = 16384 per group

LAST_EXEC_NS = None
LAST_RESULTS = None


def _build_tile(ctx: ExitStack, tc: tile.TileContext, te: dict):
    nc = tc.nc
    x_e, out_e = te["x"], te["out"]

    const = ctx.enter_context(tc.tile_pool(name="const", bufs=1))
    small = ctx.enter_context(tc.tile_pool(name="small", bufs=6))
    ps_acc = ctx.enter_context(tc.tile_pool(name="ps_acc", bufs=4, space="PSUM"))
    ps_sc = ctx.enter_context(tc.tile_pool(name="ps_sc", bufs=2, space="PSUM"))

    # ---- groupnorm stats over the [BS*G, 16384] view of x ----
    # Issued before the weight loads: the stats chain gates the first matmul.
    # Chunked DMAs so bn_stats tracks the stream instead of one 4MB barrier.
    NCHUNK = GROUP_ELEMS // 512          # bn_stats hw max free = 512
    GCH = 4
    stats_sb = const.tile([BS * G, NCHUNK, 6], F32)
    eps_sb = const.tile([BS * G, 1], F32)
    nc.vector.memset(eps_sb[:], EPS)
    # preload the Exp ACT table set off the critical path
    dummy_act = const.tile([1, 1], F32)
    nc.scalar.activation(dummy_act[:], eps_sb[0:1, :], AF.Exp)
    # stats on [128, 8192] half-group rows: full-width DMA ports (a [64, N]
    # layout would halve DMA bandwidth) and half the bn_stats calls
    HSUB = GROUP_ELEMS // 2 // 512 // GCH  # 512-wide bn_stats per DMA chunk
    stats2 = const.tile([128, GROUP_ELEMS // 2 // 512, 6], F32)
    with tc.tile_pool(name="gnx", bufs=2) as gnxp:
        for gc in range(GCH):
            gnx = gnxp.tile([128, HSUB, 512], F32, tag="gnx", name="gnx")
            in_ap = bass.AP(
                tensor=x_e,
                offset=gc * HSUB * 512,
                ap=[[C * N, BS], [GROUP_ELEMS // 2, 2 * G], [1, HSUB * 512]],
            )
            nc.sync.dma_start(out=gnx[:], in_=in_ap)
            for j in range(HSUB):
                nc.vector.bn_stats(out=stats2[:, gc * HSUB + j, :], in_=gnx[:, j, :])
    # fold half-group stats rows back to [group, 2*chunks] (both sides are
    # contiguous, single sbuf-to-sbuf DMA), then aggregate per group
    nc.gpsimd.dma_start(out=stats_sb[:], in_=stats2[:])

    # ---- constants / weights to SBUF (needed ~30us in; loads overlap stats) ----
    wqk_sb = const.tile([128, CT, 2 * C], BF16)   # w_qkv[:1024].T tiles
    wv_sb = const.tile([128, CT, C], BF16)        # w_qkv[1024:].T tiles
    wp_sb = const.tile([128, CT, C], BF16)        # w_proj.T tiles
    bq_sb = const.tile([128, CT, 1], F32)
    beff_sb = const.tile([128, CT, 1], F32)
    for kt in range(CT):
        sl = slice(kt * 128, (kt + 1) * 128)
        nc.sync.dma_start(out=wqk_sb[:, kt, :], in_=te["wqkT"][sl, :])
        nc.sync.dma_start(out=wv_sb[:, kt, :], in_=te["wvT"][sl, :])
        nc.sync.dma_start(out=wp_sb[:, kt, :], in_=te["wpT"][sl, :])
        nc.sync.dma_start(out=bq_sb[:, kt, :], in_=te["bq"][sl, :])
        nc.sync.dma_start(out=beff_sb[:, kt, :], in_=te["beff"][sl, :])
    # gamma/beta replicated per sample: [128, (s, t)] layout
    gam2 = const.tile([128, BS * CT], F32)
    bet2 = const.tile([128, BS * CT], F32)
    for s in range(BS):
        nc.sync.dma_start(
            out=gam2[:, s * CT : (s + 1) * CT],
            in_=bass.AP(tensor=te["gamma"], offset=0, ap=[[1, 128], [128, CT]]),
        )
        nc.sync.dma_start(
            out=bet2[:, s * CT : (s + 1) * CT],
            in_=bass.AP(tensor=te["beta"], offset=0, ap=[[1, 128], [128, CT]]),
        )

    mv = small.tile([BS * G, 2], F32, tag="mv")
    nc.vector.bn_aggr(out=mv[:], in_=stats_sb[:])
    # rstd = rsqrt(var + eps) via int-seed + 2 Newton steps, all on DVE —
    # keeps ACT on the single Exp table set for the whole kernel (Ln/Sqrt
    # would force table reloads).
    I32 = mybir.dt.int32
    st2 = small.tile([BS * G, 2], mybir.dt.float32r, tag="st2")
    nc.vector.tensor_copy(st2[:, 0:1], mv[:, 0:1])
    vpe = small.tile([BS * G, 1], F32, tag="vpe")
    nc.vector.tensor_scalar_add(vpe[:], mv[:, 1:2], EPS)
    hv = small.tile([BS * G, 1], F32, tag="hv")
    nc.vector.tensor_scalar_mul(hv[:], vpe[:], -0.5)
    y0 = small.tile([BS * G, 1], F32, tag="y0")
    ysh = small.tile([BS * G, 1], I32, tag="ysh")
    nc.vector.tensor_scalar(
        out=ysh[:],
        in0=vpe[:].bitcast(I32),
        scalar1=1,
        scalar2=None,
        op0=ALU.arith_shift_right,
    )
    nc.vector.tensor_scalar(
        out=y0[:].bitcast(I32),
        in0=ysh[:],
        scalar1=-1,
        scalar2=0x5F3759DF,
        op0=ALU.mult,
        op1=ALU.add,
    )
    y1 = small.tile([BS * G, 1], F32, tag="y1")
    yw = small.tile([BS * G, 1], F32, tag="yw")
    # Newton: y <- y * (1.5 - 0.5*v*y^2), twice
    nc.vector.tensor_mul(yw[:], y0[:], y0[:])
    nc.vector.tensor_mul(yw[:], yw[:], hv[:])
    nc.vector.tensor_scalar_add(yw[:], yw[:], 1.5)
    nc.vector.tensor_mul(y1[:], y0[:], yw[:])
    nc.vector.tensor_mul(yw[:], y1[:], y1[:])
    nc.vector.tensor_mul(yw[:], yw[:], hv[:])
    nc.vector.tensor_scalar_add(yw[:], yw[:], 1.5)
    nc.vector.tensor_mul(st2[:, 1:2], y1[:], yw[:])

    # broadcast group stats to channel vectors with tiny selector matmuls on
    # the (otherwise idle) PE: mvr[p, j, :] = (mean, rstd) of group g(p, j);
    # SEL comes from the host, f32r keeps the stats at ~f32 precision
    F32R = mybir.dt.float32r
    A_all = const.tile([128, BS * CT], F32)
    B_all = const.tile([128, BS * CT], F32)
    with tc.tile_pool(name="selp", bufs=1) as selp:
        sel_sb = selp.tile([BS * G, BS * CT, 128], F32R)
        nc.sync.dma_start(
            out=sel_sb[:], in_=te["sel"][:].rearrange("g (j p) -> g j p", p=128)
        )
        mvr_ps = ps_sc.tile([128, BS * CT, 2], F32, tag="sc", name="mvr_ps")
        for j in range(BS * CT):
            nc.tensor.matmul(
                mvr_ps[:, j, :],
                sel_sb[:, j, :],
                st2[:],
                start=True,
                stop=True,
            )
        # h = x*A + Bv over all (s, t): A = rstd*gamma, Bv = beta - mean*A
        nc.vector.tensor_mul(A_all[:], mvr_ps[:, :, 1], gam2[:])
        tmpA = small.tile([128, BS * CT], F32, tag="tmpA")
        nc.vector.tensor_mul(tmpA[:], mvr_ps[:, :, 0], A_all[:])
        nc.vector.tensor_sub(B_all[:], bet2[:], tmpA[:])

    # Main pools open after the gn-stats/sel pools have freed their space.
    xpool = ctx.enter_context(tc.tile_pool(name="xres", bufs=1))
    hpool = ctx.enter_context(tc.tile_pool(name="h", bufs=1))
    qkpool = ctx.enter_context(tc.tile_pool(name="qk", bufs=2))
    vtpool = ctx.enter_context(tc.tile_pool(name="vt", bufs=2))
    atpool = ctx.enter_context(tc.tile_pool(name="attn", bufs=2))
    aopool = ctx.enter_context(tc.tile_pool(name="ao", bufs=2))
    rbpool = ctx.enter_context(tc.tile_pool(name="rb", bufs=4))
    rcppool = ctx.enter_context(tc.tile_pool(name="rcps", bufs=2))
    outpool = ctx.enter_context(tc.tile_pool(name="outp", bufs=2))

    x_sb = xpool.tile([128, BS * CT, N], F32)
    for s in range(BS):
        for t in range(CT):
            nc.gpsimd.dma_start(
                out=x_sb[:, s * CT + t, :], in_=x_e[s, t * 128 : (t + 1) * 128, :]
            )

    def emit_prep(s):
        # ---- groupnorm apply -> h (bf16), then qk / vT matmuls ----
        h_sb = hpool.tile([128, CT, N], BF16, tag="h", name="h_sb")
        for t in range(CT):
            nc.vector.tensor_scalar(
                out=h_sb[:, t, :],
                in0=x_sb[:, s * CT + t, :],
                scalar1=A_all[:, s * CT + t : s * CT + t + 1],
                scalar2=B_all[:, s * CT + t : s * CT + t + 1],
                op0=ALU.mult,
                op1=ALU.add,
            )

        # ---- qk = wqkT.T @ h   ([o, n], o-tile p holds heads 2p, 2p+1) ----
        q_sb = qkpool.tile([128, CT, N], BF16, tag="q", name="q_sb")
        k_sb = qkpool.tile([128, CT, N], BF16, tag="k", name="k_sb")
        for o in range(2 * CT):
            for nh in range(NHALF):
                ps = ps_acc.tile([128, 512], F32, tag="acc", name="ps")
                for kt in range(CT):
                    nc.tensor.matmul(
                        ps[:],
                        wqk_sb[:, kt, o * 128 : (o + 1) * 128],
                        h_sb[:, kt, nh * 512 : (nh + 1) * 512],
                        start=(kt == 0),
                        stop=(kt == CT - 1),
                    )
                if o < CT:  # q channels: scale+bias fused into the copy
                    nc.vector.tensor_scalar(
                        out=q_sb[:, o, nh * 512 : (nh + 1) * 512],
                        in0=ps[:],
                        scalar1=bq_sb[:, o, :],
                        scalar2=float(D) ** -0.5,
                        op0=ALU.add,
                        op1=ALU.mult,
                    )
                else:  # k channels: plain copy (bias dropped, see header)
                    nc.vector.tensor_copy(
                        k_sb[:, o - CT, nh * 512 : (nh + 1) * 512], ps[:]
                    )

        # ---- vT = h.T @ wvT  ([m, dv] + ones column for colsum) ----
        vt_sb = vtpool.tile([128, MT, HEADS, D + 1], BF16, tag="vt")
        nc.vector.memset(vt_sb[:, :, :, D : D + 1], 1.0)
        for m in range(MT):
            ps = ps_acc.tile([128, 512], F32, tag="acc", name="ps")
            for kt in range(CT):
                nc.tensor.matmul(
                    ps[:],
                    h_sb[:, kt, m * 128 : (m + 1) * 128],
                    wv_sb[:, kt, :],
                    start=(kt == 0),
                    stop=(kt == CT - 1),
                )
            nc.vector.tensor_copy(
                vt_sb[:, m, :, 0:D], ps[:].rearrange("p (h d) -> p h d", h=HEADS)
            )

        return q_sb, k_sb, vt_sb

    def emit_attention(s, q_sb, k_sb, vt_sb):
        # ---- attention: QK/exp of pair p interleaved with AV of pair p-1 ----
        # (fills the PE gaps while ACT runs exp; ~2x denser PE stream)
        ao_sb = aopool.tile([128, CT, N], BF16, tag="ao", name="ao_sb")

        def emit_av_chunk(prev_state, m):
            p0, at0, avs0 = prev_state
            for hh in range(2):
                for nh in range(NHALF):
                    nc.tensor.matmul(
                        avs0[hh][nh][:],
                        vt_sb[:, m, 2 * p0 + hh, :],
                        at0[:, hh, m, nh * 512 : (nh + 1) * 512],
                        start=(m == 0),
                        stop=(m == MT - 1),
                    )

        def emit_normalize(prev_state):
            p0, at0, avs0 = prev_state
            for hh in range(2):
                for nh in range(NHALF):
                    nsl = slice(nh * 512, (nh + 1) * 512)
                    # custom-DVE recip misreads PSUM sources on HW: SBUF-bounce
                    cs = rcppool.tile([1, 512], F32, tag="cs", name="cs")
                    nc.vector.tensor_copy(cs[:], avs0[hh][nh][D : D + 1, :])
                    rcp = rcppool.tile([1, 512], F32, tag="rcp", name="rcp")
                    nc.vector.reciprocal_approx_fast(rcp[:], cs[:])
                    rb = rbpool.tile([64, 512], F32, tag="rb", name="rb")
                    nc.gpsimd.partition_broadcast(rb[:], rcp[:])
                    nc.vector.tensor_mul(
                        ao_sb[hh * 64 : (hh + 1) * 64, p0, nsl],
                        avs0[hh][nh][0:D, :],
                        rb[:],
                    )

        prev = None
        for p in range(HEADS // 2):
            at_pair = atpool.tile([128, 2, MT, N], BF16, tag="attn", name="at_pair")
            for m in range(MT):
                for hh in range(2):
                    base = hh * 64
                    sc = ps_sc.tile([128, N], F32, tag="sc", name="sc")
                    for nh in range(NHALF):
                        nsl = slice(nh * 512, (nh + 1) * 512)
                        nc.tensor.matmul(
                            sc[:, nsl],
                            k_sb[base : base + 64, p, m * 128 : (m + 1) * 128],
                            q_sb[base : base + 64, p, nsl],
                            start=True,
                            stop=True,
                            tile_position=(base, 0),
                        )
                    nc.scalar.activation(at_pair[:, hh, m, :], sc[:], AF.Exp)
                if prev is not None:
                    emit_av_chunk(prev, m)
            if prev is not None:
                emit_normalize(prev)
            if p < HEADS // 2 - 1:
                avs = [
                    [
                        ps_acc.tile([D + 1, 512], F32, tag="acc", name=f"av{hh}_{nh}")
                        for nh in range(NHALF)
                    ]
                    for hh in range(2)
                ]
            else:
                # drain pair accumulates in the (then idle) scores pool so the
                # acc pool frees for the next sample's qkv before normalize
                dr = [
                    ps_sc.tile([D + 1, N], F32, tag="sc", name=f"drain{hh}")
                    for hh in range(2)
                ]
                avs = [
                    [dr[hh][:, nh * 512 : (nh + 1) * 512] for nh in range(NHALF)]
                    for hh in range(2)
                ]
            prev = (p, at_pair, avs)
        for m in range(MT):
            emit_av_chunk(prev, m)
        emit_normalize(prev)
        return ao_sb

    def emit_proj(s, ao_sb):
        # ---- proj + bias + residual ----
        for t in range(CT):
            for nh in range(NHALF):
                nsl = slice(nh * 512, (nh + 1) * 512)
                ps = ps_acc.tile([128, 512], F32, tag="acc", name="ps")
                for kt in range(CT):
                    nc.tensor.matmul(
                        ps[:],
                        wp_sb[:, kt, t * 128 : (t + 1) * 128],
                        ao_sb[:, kt, nsl],
                        start=(kt == 0),
                        stop=(kt == CT - 1),
                    )
                ot = outpool.tile([128, 512], F32, tag="out")
                nc.vector.scalar_tensor_tensor(
                    out=ot[:],
                    in0=ps[:],
                    scalar=beff_sb[:, t, :],
                    in1=x_sb[:, s * CT + t, nsl],
                    op0=ALU.add,
                    op1=ALU.add,
                )
                nc.gpsimd.dma_start(
                    out=out_e[s, t * 128 : (t + 1) * 128, nsl], in_=ot[:]
                )

    # Drive: emit next sample's qkv prep between a sample's attention drain
    # and its proj, so the PE instruction stream has work while the
    # normalize (recip -> DRAM bounce -> broadcast) latency resolves.
    tiles = emit_prep(0)
    for s in range(BS):
        ao = emit_attention(s, *tiles)
        if s + 1 < BS:
            tiles = emit_prep(s + 1)
        emit_proj(s, ao)


def build_bass() -> bass.Bass:
    nc = bacc.Bacc()
    te = {
        "x": nc.declare_dram_parameter("x", [BS, C, N], F32, isOutput=False),
        "wqkT": nc.declare_dram_parameter("wqkT", [C, 2 * C], BF16, isOutput=False),
        "wvT": nc.declare_dram_parameter("wvT", [C, C], BF16, isOutput=False),
        "wpT": nc.declare_dram_parameter("wpT", [C, C], BF16, isOutput=False),
        "bq": nc.declare_dram_parameter("bq", [C, 1], F32, isOutput=False),
        "beff": nc.declare_dram_parameter("beff", [C, 1], F32, isOutput=False),
        "gamma": nc.declare_dram_parameter("gamma", [C, 1], F32, isOutput=False),
        "beta": nc.declare_dram_parameter("beta", [C, 1], F32, isOutput=False),
        "sel": nc.declare_dram_parameter(
            "sel", [BS * G, BS * CT * 128], mybir.dt.float32r, isOutput=False
        ),
        "out": nc.declare_dram_parameter("out", [BS, C, N], F32, isOutput=True),
    }
    with tile.TileContext(nc) as tc:
        with ExitStack() as ctx:
            _build_tile(ctx, tc, te)
    # Bacc defers register allocation to finalize(); run_bass_via_pjrt
    # serializes the module without calling it, so do it here.
    nc.finalize()
    return nc


def _make_sel() -> np.ndarray:
    sel = np.zeros((BS * G, BS * CT, 128), np.float32)
    for j in range(BS * CT):
        s0, t0 = j // CT, j % CT
        for p in range(128):
            sel[s0 * G + t0 * 8 + p // 16, j, p] = 1.0
    return sel.reshape(BS * G, BS * CT * 128)


def make_in_maps(inputs: dict) -> list[dict]:
    x = np.ascontiguousarray(np.asarray(inputs["x"], np.float32)).reshape(B, C, N)
    w_qkv = np.asarray(inputs["w_qkv"], np.float32)
    b_qkv = np.asarray(inputs["b_qkv"], np.float32)
    w_proj = np.asarray(inputs["w_proj"], np.float32)
    b_proj = np.asarray(inputs["b_proj"], np.float32)
    gamma = np.asarray(inputs["gamma"], np.float32)
    beta = np.asarray(inputs["beta"], np.float32)

    bf = ml_dtypes.bfloat16
    common = {
        "wqkT": np.ascontiguousarray(w_qkv[: 2 * C, :].T).astype(bf),
        "wvT": np.ascontiguousarray(w_qkv[2 * C :, :].T).astype(bf),
        "wpT": np.ascontiguousarray(w_proj.T).astype(bf),
        "bq": b_qkv[:C].reshape(C, 1).copy(),
        "beff": (b_proj + w_proj @ b_qkv[2 * C :]).reshape(C, 1).astype(np.float32),
        "gamma": gamma.reshape(C, 1).copy(),
        "beta": beta.reshape(C, 1).copy(),
        "sel": _make_sel(),
    }
    return [
        {"x": np.ascontiguousarray(x[i * BS : (i + 1) * BS]), **common}
        for i in range(NCORES)
    ]


def kernel(**inputs) -> np.ndarray:
    global LAST_EXEC_NS, LAST_RESULTS
    nc = build_bass()
    in_maps = make_in_maps(inputs)
    res = run_bass_kernel_spmd(nc, in_maps, list(range(NCORES)))
    LAST_RESULTS = res
    LAST_EXEC_NS = res.exec_time_ns
    out = np.concatenate([np.asarray(res.results[i]["out"]) for i in range(NCORES)], 0)
    return out.reshape(B, C, H, W).astype(np.float32)


# revision 49
# speedup vs baseline: 1.1799x; 1.0606x over previous
"""Trainium2 Bass kernel: AttentionBlock (GroupNorm + 1x1-conv QKV + MHA + proj + residual).

Data-parallel over batch: 16 samples -> 8 NeuronCores x 2 samples. Each core
runs the whole block locally (attention is per-sample, no collectives); the
host shards inputs and concatenates the 8 output shards.

Math notes (exact rewrites, not approximations):
  - scores are computed transposed, S^T[m,n] = sum_d k[d,m] q'[d,n] with
    q' = (q + b_q) * d^-0.5. The k-bias adds a column-constant to S^T which
    softmax cancels, so it is dropped.
  - softmax denominator comes from a ones-column appended to v^T in the
    attn@v matmul (row 64 of the [65, n] output accumulates colsum(exp S^T)).
  - v-bias: attn rows sum to 1, so  attn @ (Wv h + bv) = attn @ Wv h + bv;
    the bv term is folded into the proj bias on the host:
    beff = b_proj + w_proj @ bv.
"""

import os
import sys
import types
from contextlib import ExitStack

import ml_dtypes
import numpy as np

# If BASS_TRACE is set but this container's antenv lacks the NTFF hook
# module, bass_utils' trace path would crash on import; give it a null
# hook so tracing degrades gracefully instead.
try:
    import antenv.axon_hooks  # noqa: F401
except Exception:  # pragma: no cover
    try:
        import antenv

        _hookmod = types.ModuleType("antenv.axon_hooks")
        _hook = [None]
        _hookmod.set_axon_ntff_profile_hook = lambda h: _hook.__setitem__(0, h)
        _hookmod.get_axon_ntff_profile_hook = lambda: _hook[0]
        sys.modules["antenv.axon_hooks"] = _hookmod
        antenv.axon_hooks = _hookmod
    except Exception:
        pass

import concourse.bass as bass
import concourse.tile as tile
from concourse import bacc
from concourse import mybir
from concourse.bass_utils import run_bass_kernel_spmd

F32 = mybir.dt.float32
BF16 = mybir.dt.bfloat16
AF = mybir.ActivationFunctionType
ALU = mybir.AluOpType

# Problem dims (hardcoded per spec: x [16, 512, 32, 32] f32)
B, C, H, W = 16, 512, 32, 32
N = H * W                # 1024 spatial positions
NCORES = 8
BS = B // NCORES         # 2 samples per core
G = 32                   # groupnorm groups
HEADS = 8
D = C // HEADS           # 64
CT = C // 128            # 4 channel tiles
MT = N // 128            # 8 m-tiles (spatial, attention contraction)
NHALF = 2                # n split in halves of 512 (psum bank limit)
EPS = 1e-5
GROUP_ELEMS = (C // G) * N   # 16 ch * 1024 = 16384 per group

LAST_EXEC_NS = None
LAST_RESULTS = None


def _build_tile(ctx: ExitStack, tc: tile.TileContext, te: dict):
    nc = tc.nc
    x_e, out_e = te["x"], te["out"]

    const = ctx.enter_context(tc.tile_pool(name="const", bufs=1))
    small = ctx.enter_context(tc.tile_pool(name="small", bufs=6))
    ps_acc = ctx.enter_context(tc.tile_pool(name="ps_acc", bufs=4, space="PSUM"))
    ps_sc = ctx.enter_context(tc.tile_pool(name="ps_sc", bufs=2, space="PSUM"))

    # ---- groupnorm stats over the [BS*G, 16384] view of x ----
    # Issued before the weight loads: the stats chain gates the first matmul.
    # Chunked DMAs so bn_stats tracks the stream instead of one 4MB barrier.
    NCHUNK = GROUP_ELEMS // 512          # bn_stats hw max free = 512
    GCH = 4
    stats_sb = const.tile([BS * G, NCHUNK, 6], F32)
    eps_sb = const.tile([BS * G, 1], F32)
    nc.vector.memset(eps_sb[:], EPS)
    # preload the Exp ACT table set off the critical path
    dummy_act = const.tile([1, 1], F32)
    nc.scalar.activation(dummy_act[:], eps_sb[0:1, :], AF.Exp)
    # stats on [128, 8192] half-group rows: full-width DMA ports (a [64, N]
    # layout would halve DMA bandwidth) and half the bn_stats calls
    HSUB = GROUP_ELEMS // 2 // 512 // GCH  # 512-wide bn_stats per DMA chunk
    stats2 = const.tile([128, GROUP_ELEMS // 2 // 512, 6], F32)
    with tc.tile_pool(name="gnx", bufs=2) as gnxp:
        for gc in range(GCH):
            gnx = gnxp.tile([128, HSUB, 512], F32, tag="gnx", name="gnx")
            in_ap = bass.AP(
                tensor=x_e,
                offset=gc * HSUB * 512,
                ap=[[C * N, BS], [GROUP_ELEMS // 2, 2 * G], [1, HSUB * 512]],
            )
            nc.sync.dma_start(out=gnx[:], in_=in_ap)
            for j in range(HSUB):
                nc.vector.bn_stats(out=stats2[:, gc * HSUB + j, :], in_=gnx[:, j, :])
    # fold half-group stats rows back to [group, 2*chunks] (both sides are
    # contiguous, single sbuf-to-sbuf DMA), then aggregate per group
    nc.gpsimd.dma_start(out=stats_sb[:], in_=stats2[:])

    # ---- constants / weights to SBUF (needed ~30us in; loads overlap stats) ----
    wqk_sb = const.tile([128, CT, 2 * C], BF16)   # w_qkv[:1024].T tiles
    wv_sb = const.tile([128, CT, C], BF16)        # w_qkv[1024:].T tiles
    wp_sb = const.tile([128, CT, C], BF16)        # w_proj.T tiles
    bq_sb = const.tile([128, CT, 1], F32)
    beff_sb = const.tile([128, CT, 1], F32)
    for kt in range(CT):
        sl = slice(kt * 128, (kt + 1) * 128)
        nc.sync.dma_start(out=wqk_sb[:, kt, :], in_=te["wqkT"][sl, :])
        nc.sync.dma_start(out=wv_sb[:, kt, :], in_=te["wvT"][sl, :])
        nc.sync.dma_start(out=wp_sb[:, kt, :], in_=te["wpT"][sl, :])
        nc.sync.dma_start(out=bq_sb[:, kt, :], in_=te["bq"][sl, :])
        nc.sync.dma_start(out=beff_sb[:, kt, :], in_=te["beff"][sl, :])
    # gamma/beta replicated per sample: [128, (s, t)] layout
    gam2 = const.tile([128, BS * CT], F32)
    bet2 = const.tile([128, BS * CT], F32)
    for s in range(BS):
        nc.sync.dma_start(
            out=gam2[:, s * CT : (s + 1) * CT],
            in_=bass.AP(tensor=te["gamma"], offset=0, ap=[[1, 128], [128, CT]]),
        )
        nc.sync.dma_start(
            out=bet2[:, s * CT : (s + 1) * CT],
            in_=bass.AP(tensor=te["beta"], offset=0, ap=[[1, 128], [128, CT]]),
        )

    mv = small.tile([BS * G, 2], F32, tag="mv")
    nc.vector.bn_aggr(out=mv[:], in_=stats_sb[:])
    # rstd = rsqrt(var + eps) via int-seed + 2 Newton steps, all on DVE —
    # keeps ACT on the single Exp table set for the whole kernel (Ln/Sqrt
    # would force table reloads).
    I32 = mybir.dt.int32
    st2 = small.tile([BS * G, 2], mybir.dt.float32r, tag="st2")
    nc.vector.tensor_copy(st2[:, 0:1], mv[:, 0:1])
    vpe = small.tile([BS * G, 1], F32, tag="vpe")
    nc.vector.tensor_scalar_add(vpe[:], mv[:, 1:2], EPS)
    hv = small.tile([BS * G, 1], F32, tag="hv")
    nc.vector.tensor_scalar_mul(hv[:], vpe[:], -0.5)
    y0 = small.tile([BS * G, 1], F32, tag="y0")
    ysh = small.tile([BS * G, 1], I32, tag="ysh")
    nc.vector.tensor_scalar(
        out=ysh[:],
        in0=vpe[:].bitcast(I32),
        scalar1=1,
        scalar2=None,
        op0=ALU.arith_shift_right,
    )
    nc.vector.tensor_scalar(
        out=y0[:].bitcast(I32),
        in0=ysh[:],
        scalar1=-1,
        scalar2=0x5F3759DF,
        op0=ALU.mult,
        op1=ALU.add,
    )
    y1 = small.tile([BS * G, 1], F32, tag="y1")
    yw = small.tile([BS * G, 1], F32, tag="yw")
    # Newton: y <- y * (1.5 - 0.5*v*y^2), twice
    nc.vector.tensor_mul(yw[:], y0[:], y0[:])
    nc.vector.tensor_mul(yw[:], yw[:], hv[:])
    nc.vector.tensor_scalar_add(yw[:], yw[:], 1.5)
    nc.vector.tensor_mul(y1[:], y0[:], yw[:])
    nc.vector.tensor_mul(yw[:], y1[:], y1[:])
    nc.vector.tensor_mul(yw[:], yw[:], hv[:])
    nc.vector.tensor_scalar_add(yw[:], yw[:], 1.5)
    nc.vector.tensor_mul(st2[:, 1:2], y1[:], yw[:])

    # broadcast group stats to channel vectors with tiny selector matmuls on
    # the (otherwise idle) PE: mvr[p, j, :] = (mean, rstd) of group g(p, j);
    # SEL comes from the host, f32r keeps the stats at ~f32 precision
    F32R = mybir.dt.float32r
    A_all = const.tile([128, BS * CT], F32)
    B_all = const.tile([128, BS * CT], F32)
    with tc.tile_pool(name="selp", bufs=1) as selp:
        sel_sb = selp.tile([BS * G, BS * CT, 128], F32R)
        nc.sync.dma_start(
            out=sel_sb[:], in_=te["sel"][:].rearrange("g (j p) -> g j p", p=128)
        )
        mvr_ps = ps_sc.tile([128, BS * CT, 2], F32, tag="sc", name="mvr_ps")
        for j in range(BS * CT):
            nc.tensor.matmul(
                mvr_ps[:, j, :],
                sel_sb[:, j, :],
                st2[:],
                start=True,
                stop=True,
            )
        # h = x*A + Bv over all (s, t): A = rstd*gamma, Bv = beta - mean*A
        nc.vector.tensor_mul(A_all[:], mvr_ps[:, :, 1], gam2[:])
        tmpA = small.tile([128, BS * CT], F32, tag="tmpA")
        nc.vector.tensor_mul(tmpA[:], mvr_ps[:, :, 0], A_all[:])
        nc.vector.tensor_sub(B_all[:], bet2[:], tmpA[:])

    # Main pools open after the gn-stats/sel pools have freed their space.
    xpool = ctx.enter_context(tc.tile_pool(name="xres", bufs=1))
    hpool = ctx.enter_context(tc.tile_pool(name="h", bufs=1))
    qkpool = ctx.enter_context(tc.tile_pool(name="qk", bufs=2))
    vtpool = ctx.enter_context(tc.tile_pool(name="vt", bufs=2))
    atpool = ctx.enter_context(tc.tile_pool(name="attn", bufs=2))
    aopool = ctx.enter_context(tc.tile_pool(name="ao", bufs=2))
    rbpool = ctx.enter_context(tc.tile_pool(name="rb", bufs=4))
    rcppool = ctx.enter_context(tc.tile_pool(name="rcps", bufs=2))
    outpool = ctx.enter_context(tc.tile_pool(name="outp", bufs=2))

    x_sb = xpool.tile([128, BS * CT, N], F32)
    for s in range(BS):
        for t in range(CT):
            nc.gpsimd.dma_start(
                out=x_sb[:, s * CT + t, :], in_=x_e[s, t * 128 : (t + 1) * 128, :]
            )

    def emit_prep(s):
        # ---- groupnorm apply -> h (bf16), then qk / vT matmuls ----
        h_sb = hpool.tile([128, CT, N], BF16, tag="h", name="h_sb")
        for t in range(CT):
            if t % 2 == 0:  # split h-apply across ACT and DVE
                nc.scalar.activation(
                    h_sb[:, t, :],
                    x_sb[:, s * CT + t, :],
                    AF.Identity,
                    bias=B_all[:, s * CT + t : s * CT + t + 1],
                    scale=A_all[:, s * CT + t : s * CT + t + 1],
                )
            else:
                nc.vector.tensor_scalar(
                    out=h_sb[:, t, :],
                    in0=x_sb[:, s * CT + t, :],
                    scalar1=A_all[:, s * CT + t : s * CT + t + 1],
                    scalar2=B_all[:, s * CT + t : s * CT + t + 1],
                    op0=ALU.mult,
                    op1=ALU.add,
                )

        # ---- qk = wqkT.T @ h   ([o, n], o-tile p holds heads 2p, 2p+1) ----
        q_sb = qkpool.tile([128, CT, N], BF16, tag="q", name="q_sb")
        k_sb = qkpool.tile([128, CT, N], BF16, tag="k", name="k_sb")
        for o in range(2 * CT):
            for nh in range(NHALF):
                ps = ps_acc.tile([128, 512], F32, tag="acc", name="ps")
                for kt in range(CT):
                    nc.tensor.matmul(
                        ps[:],
                        wqk_sb[:, kt, o * 128 : (o + 1) * 128],
                        h_sb[:, kt, nh * 512 : (nh + 1) * 512],
                        start=(kt == 0),
                        stop=(kt == CT - 1),
                    )
                if o < CT:  # q channels: scale+bias fused into the copy
                    nc.vector.tensor_scalar(
                        out=q_sb[:, o, nh * 512 : (nh + 1) * 512],
                        in0=ps[:],
                        scalar1=bq_sb[:, o, :],
                        scalar2=float(D) ** -0.5,
                        op0=ALU.add,
                        op1=ALU.mult,
                    )
                else:  # k channels: plain copy (bias dropped, see header)
                    nc.vector.tensor_copy(
                        k_sb[:, o - CT, nh * 512 : (nh + 1) * 512], ps[:]
                    )

        # ---- vT = h.T @ wvT  ([m, dv] + ones column for colsum) ----
        vt_sb = vtpool.tile([128, MT, HEADS, D + 1], BF16, tag="vt")
        nc.vector.memset(vt_sb[:, :, :, D : D + 1], 1.0)
        for m in range(MT):
            ps = ps_acc.tile([128, 512], F32, tag="acc", name="ps")
            for kt in range(CT):
                nc.tensor.matmul(
                    ps[:],
                    h_sb[:, kt, m * 128 : (m + 1) * 128],
                    wv_sb[:, kt, :],
                    start=(kt == 0),
                    stop=(kt == CT - 1),
                )
            nc.vector.tensor_copy(
                vt_sb[:, m, :, 0:D], ps[:].rearrange("p (h d) -> p h d", h=HEADS)
            )

        return q_sb, k_sb, vt_sb

    def emit_attention(s, q_sb, k_sb, vt_sb):
        # ---- attention: QK/exp of pair p interleaved with AV of pair p-1 ----
        # (fills the PE gaps while ACT runs exp; ~2x denser PE stream)
        ao_sb = aopool.tile([128, CT, N], BF16, tag="ao", name="ao_sb")

        def emit_av_chunk(prev_state, m):
            p0, at0, avs0 = prev_state
            for hh in range(2):
                for nh in range(NHALF):
                    nc.tensor.matmul(
                        avs0[hh][nh][:],
                        vt_sb[:, m, 2 * p0 + hh, :],
                        at0[:, hh, m, nh * 512 : (nh + 1) * 512],
                        start=(m == 0),
                        stop=(m == MT - 1),
                    )

        def emit_normalize(prev_state):
            p0, at0, avs0 = prev_state
            for hh in range(2):
                for nh in range(NHALF):
                    nsl = slice(nh * 512, (nh + 1) * 512)
                    # custom-DVE recip misreads PSUM sources on HW: SBUF-bounce
                    cs = rcppool.tile([1, 512], F32, tag="cs", name="cs")
                    nc.vector.tensor_copy(cs[:], avs0[hh][nh][D : D + 1, :])
                    rcp = rcppool.tile([1, 512], F32, tag="rcp", name="rcp")
                    nc.vector.reciprocal_approx_fast(rcp[:], cs[:])
                    rb = rbpool.tile([64, 512], F32, tag="rb", name="rb")
                    nc.gpsimd.partition_broadcast(rb[:], rcp[:])
                    nc.vector.tensor_mul(
                        ao_sb[hh * 64 : (hh + 1) * 64, p0, nsl],
                        avs0[hh][nh][0:D, :],
                        rb[:],
                    )

        prev = None
        for p in range(HEADS // 2):
            at_pair = atpool.tile([128, 2, MT, N], BF16, tag="attn", name="at_pair")
            for m in range(MT):
                for hh in range(2):
                    base = hh * 64
                    sc = ps_sc.tile([128, N], F32, tag="sc", name="sc")
                    for nh in range(NHALF):
                        nsl = slice(nh * 512, (nh + 1) * 512)
                        nc.tensor.matmul(
                            sc[:, nsl],
                            k_sb[base : base + 64, p, m * 128 : (m + 1) * 128],
                            q_sb[base : base + 64, p, nsl],
                            start=True,
                            stop=True,
                            tile_position=(base, 0),
                        )
                    nc.scalar.activation(at_pair[:, hh, m, :], sc[:], AF.Exp)
                if prev is not None:
                    emit_av_chunk(prev, m)
            if prev is not None:
                emit_normalize(prev)
            if p < HEADS // 2 - 1:
                avs = [
                    [
                        ps_acc.tile([D + 1, 512], F32, tag="acc", name=f"av{hh}_{nh}")
                        for nh in range(NHALF)
                    ]
                    for hh in range(2)
                ]
            else:
                # drain pair accumulates in the (then idle) scores pool so the
                # acc pool frees for the next sample's qkv before normalize
                dr = [
                    ps_sc.tile([D + 1, N], F32, tag="sc", name=f"drain{hh}")
                    for hh in range(2)
                ]
                avs = [
                    [dr[hh][:, nh * 512 : (nh + 1) * 512] for nh in range(NHALF)]
                    for hh in range(2)
                ]
            prev = (p, at_pair, avs)
        for m in range(MT):
            emit_av_chunk(prev, m)
        emit_normalize(prev)
        return ao_sb

    def emit_proj(s, ao_sb):
        # ---- proj + bias + residual, two waves of 4 open psum groups ----
        # kt=0..2 partials need only pairs 0-2's ao, so they run while the
        # drain pair's normalize chain resolves; kt=3 closes each group.
        for wave in range(2):
            pss = []
            for t in range(wave * 2, wave * 2 + 2):
                for nh in range(NHALF):
                    nsl = slice(nh * 512, (nh + 1) * 512)
                    ps = ps_acc.tile(
                        [128, 512], F32, tag="acc", name=f"pj{t}_{nh}"
                    )
                    pss.append((t, nh, nsl, ps))
                    for kt in range(CT - 1):
                        nc.tensor.matmul(
                            ps[:],
                            wp_sb[:, kt, t * 128 : (t + 1) * 128],
                            ao_sb[:, kt, nsl],
                            start=(kt == 0),
                            stop=False,
                        )
            for t, nh, nsl, ps in pss:
                nc.tensor.matmul(
                    ps[:],
                    wp_sb[:, CT - 1, t * 128 : (t + 1) * 128],
                    ao_sb[:, CT - 1, nsl],
                    start=False,
                    stop=True,
                )
                ot = outpool.tile([128, 512], F32, tag="out", name="ot")
                nc.vector.scalar_tensor_tensor(
                    out=ot[:],
                    in0=ps[:],
                    scalar=beff_sb[:, t, :],
                    in1=x_sb[:, s * CT + t, nsl],
                    op0=ALU.add,
                    op1=ALU.add,
                )
                nc.gpsimd.dma_start(
                    out=out_e[s, t * 128 : (t + 1) * 128, nsl], in_=ot[:]
                )

    # Drive: emit next sample's qkv prep between a sample's attention drain
    # and its proj, so the PE instruction stream has work while the
    # normalize (recip -> DRAM bounce -> broadcast) latency resolves.
    tiles = emit_prep(0)
    for s in range(BS):
        ao = emit_attention(s, *tiles)
        if s + 1 < BS:
            tiles = emit_prep(s + 1)
        emit_proj(s, ao)


def build_bass() -> bass.Bass:
    nc = bacc.Bacc()
    te = {
        "x": nc.declare_dram_parameter("x", [BS, C, N], F32, isOutput=False),
        "wqkT": nc.declare_dram_parameter("wqkT", [C, 2 * C], BF16, isOutput=False),
        "wvT": nc.declare_dram_parameter("wvT", [C, C], BF16, isOutput=False),
        "wpT": nc.declare_dram_parameter("wpT", [C, C], BF16, isOutput=False),
        "bq": nc.declare_dram_parameter("bq", [C, 1], F32, isOutput=False),
        "beff": nc.declare_dram_parameter("beff", [C, 1], F32, isOutput=False),
        "gamma": nc.declare_dram_parameter("gamma", [C, 1], F32, isOutput=False),
        "beta": nc.declare_dram_parameter("beta", [C, 1], F32, isOutput=False),
        "sel": nc.declare_dram_parameter(
            "sel", [BS * G, BS * CT * 128], mybir.dt.float32r, isOutput=False
        ),
        "out": nc.declare_dram_parameter("out", [BS, C, N], F32, isOutput=True),
    }
    with tile.TileContext(nc) as tc:
        with ExitStack() as ctx:
            _build_tile(ctx, tc, te)
    # Bacc defers register allocation to finalize(); run_bass_via_pjrt
    # serializes the module without calling it, so do it here.
    nc.finalize()
    return nc


def _make_sel() -> np.ndarray:
    sel = np.zeros((BS * G, BS * CT, 128), np.float32)
    for j in range(BS * CT):
        s0, t0 = j // CT, j % CT
        for p in range(128):
            sel[s0 * G + t0 * 8 + p // 16, j, p] = 1.0
    return sel.reshape(BS * G, BS * CT * 128)


def make_in_maps(inputs: dict) -> list[dict]:
    x = np.ascontiguousarray(np.asarray(inputs["x"], np.float32)).reshape(B, C, N)
    w_qkv = np.asarray(inputs["w_qkv"], np.float32)
    b_qkv = np.asarray(inputs["b_qkv"], np.float32)
    w_proj = np.asarray(inputs["w_proj"], np.float32)
    b_proj = np.asarray(inputs["b_proj"], np.float32)
    gamma = np.asarray(inputs["gamma"], np.float32)
    beta = np.asarray(inputs["beta"], np.float32)

    bf = ml_dtypes.bfloat16
    common = {
        "wqkT": np.ascontiguousarray(w_qkv[: 2 * C, :].T).astype(bf),
        "wvT": np.ascontiguousarray(w_qkv[2 * C :, :].T).astype(bf),
        "wpT": np.ascontiguousarray(w_proj.T).astype(bf),
        "bq": b_qkv[:C].reshape(C, 1).copy(),
        "beff": (b_proj + w_proj @ b_qkv[2 * C :]).reshape(C, 1).astype(np.float32),
        "gamma": gamma.reshape(C, 1).copy(),
        "beta": beta.reshape(C, 1).copy(),
        "sel": _make_sel(),
    }
    return [
        {"x": np.ascontiguousarray(x[i * BS : (i + 1) * BS]), **common}
        for i in range(NCORES)
    ]


def kernel(**inputs) -> np.ndarray:
    global LAST_EXEC_NS, LAST_RESULTS
    nc = build_bass()
    in_maps = make_in_maps(inputs)
    res = run_bass_kernel_spmd(nc, in_maps, list(range(NCORES)))
    LAST_RESULTS = res
    LAST_EXEC_NS = res.exec_time_ns
    out = np.concatenate([np.asarray(res.results[i]["out"]) for i in range(NCORES)], 0)
    return out.reshape(B, C, H, W).astype(np.float32)
